# revision 10
# baseline (speedup 1.0000x reference)
"""MoLoRA (top-2 MoE LoRA routing) Trainium2 kernel — fp16 data-path version.

Full inputs -> shard tokens across 8 NeuronCores -> Bass/Tile kernel per core
-> gather full output.

Math (per token):
  logits = silu(x @ W1 + b1) @ W2 + b2
  top-2 renormalized softmax weights: w_top1 = sigmoid(l1 - l2),
  w_top2 = 1 - w_top1 (exact identity — no exp/renorm needed).
  combined = sum_e w_e * (x @ A_e @ B_e) * 2.0 ; out = base + combined.

Kernel strategy per core (2048 tokens, fp16 on the wire):
  - x is pre-transposed on the HOST into xT tiles [128 D-part, KD, TT] so no
    on-chip transposes are needed; contractions over D run at full PE rate.
  -

    Variable tile sizes (256,256,512,512,512 tokens): small leading tiles get
    compute and output stores started early so HBM stays saturated; DMA rings
    are specialized (sync: A,W1,xT loads; scalar: B,base loads; gpsimd:
    packed small constants + output stores).
  - Router mm1 in token-on-free layout; logits are produced directly
    token-major by a second matmul with hs chunks as the stationary operand.
  - Top-2 weights via max / masked-second-max / sigmoid(delta).
  - Selected-expert weights expand to the stacked expert-rank dim [80] with a
    0/1 matmul; lowT = A_all^T @ xT is scaled by them; combined output is
    lsc^T @ B_all (B pre-scaled by 2.0 on host) + base, written as fp16 and
    widened to fp32 on the host. The +base epilogue alternates DVE-direct
    adds with ACT-copy + DVE-fast-add to balance engine load.
"""
import sys

for _p in ("/opt/trn_rl_repo",):
    if _p not in sys.path:
        sys.path.insert(0, _p)

import numpy as np
from contextlib import ExitStack

import concourse.bass as bass
import concourse.tile as tile
from concourse import bacc, mybir
from concourse.bass_utils import run_bass_kernel_spmd

FP = mybir.dt.float32
F16 = mybir.dt.float16
NEG_BIG = -1e30

N_CORES = 8
B_, S, D = 4, 4096, 2048
E, R, H = 5, 16, 256
SCALING = 32.0 / 16.0
TOK = (B_ * S) // N_CORES
TTS = (256, 512, 512, 512, 256)
assert sum(TTS) == TOK


def _bcast(ap, n):
    """Append a 0-stride free dim of size n to an AP (read broadcast)."""
    a = ap.copy()
    a.ap = a.ap + [[0, n]]
    return a


def _build_nc(TOK=TOK, D=D, H=H, E=E, R=R, n_cores=N_CORES):
    from concourse.alu_op_type import AluOpType as A

    KD = D // 128
    KH = H // 128
    M = E * R
    ND = D // 512
    TTMAX = max(TTS)

    nc = bacc.Bacc("TRN2", num_devices=n_cores, debug=False)

    xt_d = nc.dram_tensor("xt", [TOK * D], F16, kind="ExternalInput")
    base_d = nc.dram_tensor("base", [TOK * D], F16, kind="ExternalInput")
    a_d = nc.dram_tensor("a_all", [128, KD * M], F16, kind="ExternalInput")
    b_d = nc.dram_tensor("b_all", [M, D], F16, kind="ExternalInput")
    w1_d = nc.dram_tensor("w1", [128, KD * H], F16, kind="ExternalInput")
    # packed small constants: f32 [128, KH + maxNCH*E] = b1 | b2-broadcast
    NCHM = TTMAX // 128
    sm32_d = nc.dram_tensor("sm32", [128, KH + NCHM * E], FP, kind="ExternalInput")
    # packed small constants: f16 [128, KH*E + M + 128] = w2 | e80 | ident
    sm16_d = nc.dram_tensor(
        "sm16", [128, KH * E + M + 128], F16, kind="ExternalInput"
    )
    out_d = nc.dram_tensor("out", [TOK * D], F16, kind="ExternalOutput")

    with tile.TileContext(nc) as tc, ExitStack() as ctx:
        const = ctx.enter_context(tc.tile_pool(name="const", bufs=1))
        xt_pool = ctx.enter_context(tc.tile_pool(name="xt", bufs=4))
        base_pool = ctx.enter_context(tc.tile_pool(name="basep", bufs=3))
        out_pool = ctx.enter_context(tc.tile_pool(name="outp", bufs=6))
        cmb_pool = ctx.enter_context(tc.tile_pool(name="cmb", bufs=4))
        zs_pool = ctx.enter_context(tc.tile_pool(name="zs", bufs=2))
        sm_pool = ctx.enter_context(tc.tile_pool(name="sm", bufs=2))
        lsc_pool = ctx.enter_context(tc.tile_pool(name="lsc", bufs=2))

        ps_h = ctx.enter_context(tc.tile_pool(name="ps_h", bufs=2, space="PSUM"))
        ps_lg = ctx.enter_context(tc.tile_pool(name="ps_lg", bufs=1, space="PSUM"))
        ps_vw = ctx.enter_context(tc.tile_pool(name="ps_vw", bufs=1, space="PSUM"))
        ps_low = ctx.enter_context(tc.tile_pool(name="ps_low", bufs=1, space="PSUM"))
        ps_out = ctx.enter_context(tc.tile_pool(name="ps_out", bufs=3, space="PSUM"))

        # sync ring order: W1-half0, xt0, W1-half1, A, xt1, ...
        w1_sb = const.tile([128, KD, H], F16)
        a_sb = const.tile([128, KD, M], F16)

        def emit_w1_half(h):
            nc.sync.dma_start(
                w1_sb[:, :, h * 128 : (h + 1) * 128],
                w1_d.ap()
                .rearrange("p (k h) -> p k h", h=H)[:, :, h * 128 : (h + 1) * 128],
            )

        def emit_a():
            nc.sync.dma_start(
                a_sb[:], a_d.ap().rearrange("p (k m) -> p k m", m=M)
            )
        # scalar ring: B then base tiles
        bb_sb = const.tile([M, D], F16)
        nc.scalar.dma_start(bb_sb[:], b_d.ap())
        # gpsimd ring: packed smalls, then output stores
        sm32_sb = const.tile([128, KH + NCHM * E], FP)
        nc.gpsimd.dma_start(sm32_sb[:], sm32_d.ap())
        sm16_sb = const.tile([128, KH * E + M + 128], F16)
        nc.gpsimd.dma_start(sm16_sb[:], sm16_d.ap())

        b1_sb = sm32_sb[:, 0:KH]
        b2b_full = sm32_sb[:, KH : KH + NCHM * E]
        w2_sb = sm16_sb[:, 0 : KH * E].rearrange("p (k e) -> p k e", e=E)
        e80_sb = sm16_sb[0:E, KH * E : KH * E + M]
        ident = sm16_sb[:, KH * E + M :]

        def emit_loads(t, off, tt):
            nch = tt // 128
            xt_sb = xt_pool.tile([128, KD, tt], F16, tag="xt_sb", name="xt_sb")
            nc.sync.dma_start(
                xt_sb[:],
                xt_d.ap()[off * D : (off + tt) * D].rearrange(
                    "(p k j) -> p k j", p=128, k=KD
                ),
            )
            base_sb = base_pool.tile(
                [128, nch, D], F16, tag="base_sb", name="base_sb"
            )
            nc.scalar.dma_start(
                base_sb[:],
                base_d.ap()[off * D : (off + tt) * D].rearrange(
                    "(p c d) -> p c d", p=128, c=nch
                ),
            )
            return xt_sb, base_sb

        def emit_router(t, tt, xt_sb):
            nch = tt // 128
            # mm1 h-outer (h=0 usable as soon as W1-half0 + xt land)
            h_ps = [
                ps_h.tile([128, tt], FP, tag="hps", name=f"h_ps{h}")
                for h in range(KH)
            ]
            for h in range(KH):
                for k in range(KD):
                    nc.tensor.matmul(
                        h_ps[h][:],
                        w1_sb[:, k, h * 128 : (h + 1) * 128],
                        xt_sb[:, k, :],
                        start=(k == 0),
                        stop=(k == KD - 1),
                    )
            low_ps = ps_low.tile([M, tt], FP, tag="low")
            for k in range(KD):
                nc.tensor.matmul(
                    low_ps[:],
                    a_sb[:, k, :],
                    xt_sb[:, k, :],
                    start=(k == 0),
                    stop=(k == KD - 1),
                )

            # silu(h + b1) = (h+b1) * sigmoid(h+b1): sg on ACT, fused mult+bias
            # on DVE via scalar_tensor_tensor
            sg_sb = zs_pool.tile([128, KH, tt], F16, tag="sg", name="sg_sb")
            hs_sb = zs_pool.tile([128, KH, tt], F16, tag="hs", name="hs_sb")
            for h in range(KH):
                nc.scalar.activation(
                    sg_sb[:, h, :], h_ps[h][:],
                    mybir.ActivationFunctionType.Sigmoid,
                    bias=b1_sb[:, h : h + 1], scale=1.0,
                )
                nc.vector.scalar_tensor_tensor(
                    hs_sb[:, h, :], h_ps[h][:], b1_sb[:, h : h + 1],
                    sg_sb[:, h, :], op0=A.add, op1=A.mult,
                )

            # logits token-major: lg[tok, e] = sum_h hs[:,h,tokblk]^T @ W2[h]
            lg_ps = ps_lg.tile([128, nch, E], FP, tag="lg")
            for c in range(nch):
                for h in range(KH):
                    nc.tensor.matmul(
                        lg_ps[:, c, :],
                        hs_sb[:, h, c * 128 : (c + 1) * 128],
                        w2_sb[:, h, :],
                        start=(h == 0),
                        stop=(h == KH - 1),
                    )

            # top-2 weights: w1 = sigmoid(m1-m2) for argmax, 1-w1 for argmax2
            b2b_sb = b2b_full[:, 0 : nch * E].rearrange("p (c e) -> p c e", e=E)
            Ls = sm_pool.tile([128, nch, E], FP, tag="Ls")
            nc.vector.tensor_tensor(Ls[:], lg_ps[:], b2b_sb, A.add)
            m1r = sm_pool.tile([128, nch], FP, tag="m1r")
            nc.vector.tensor_reduce(
                m1r[:], Ls[:], axis=mybir.AxisListType.X, op=A.max
            )
            eq = sm_pool.tile([128, nch, E], FP, tag="eq")
            nc.vector.tensor_tensor(
                eq[:], Ls[:], _bcast(m1r[:], E), A.is_equal
            )
            mk = sm_pool.tile([128, nch, E], FP, tag="mk")
            nc.vector.scalar_tensor_tensor(
                mk[:], eq[:], NEG_BIG, Ls[:], op0=A.mult, op1=A.add
            )
            m2r = sm_pool.tile([128, nch], FP, tag="m2r")
            nc.vector.tensor_reduce(
                m2r[:], mk[:], axis=mybir.AxisListType.X, op=A.max
            )
            delta = sm_pool.tile([128, nch], FP, tag="delta")
            nc.vector.tensor_tensor(delta[:], m1r[:], m2r[:], A.subtract)
            s_sg = sm_pool.tile([128, nch], FP, tag="s_sg")
            nc.scalar.activation(
                s_sg[:], delta[:], mybir.ActivationFunctionType.Sigmoid
            )
            u1 = sm_pool.tile([128, nch], FP, tag="u1")
            nc.vector.tensor_scalar(
                u1[:], s_sg[:], -1.0, 1.0, op0=A.mult, op1=A.add
            )
            u2 = sm_pool.tile([128, nch], FP, tag="u2")
            nc.vector.tensor_scalar(
                u2[:], s_sg[:], 2.0, -1.0, op0=A.mult, op1=A.add
            )
            ge2 = sm_pool.tile([128, nch, E], FP, tag="ge2")
            nc.vector.tensor_tensor(
                ge2[:], Ls[:], _bcast(m2r[:], E), A.is_ge
            )
            t1 = sm_pool.tile([128, nch, E], FP, tag="t1")
            nc.vector.tensor_tensor(t1[:], ge2[:], _bcast(u1[:], E), A.mult)
            t2 = sm_pool.tile([128, nch, E], FP, tag="t2")
            nc.vector.tensor_tensor(t2[:], eq[:], _bcast(u2[:], E), A.mult)
            v = sm_pool.tile([128, nch, E], F16, tag="v")
            nc.vector.tensor_tensor(v[:], t1[:], t2[:], A.add)

            # expand weights to stacked expert-rank dim: vT [E,tt] -> [M,tt]
            vt_ps = ps_vw.tile([E, tt], F16, tag="vw", name="vt_ps")
            for c in range(nch):
                nc.tensor.transpose(
                    vt_ps[:, c * 128 : (c + 1) * 128], v[:, c, :], ident
                )
            vt_sb = sm_pool.tile([E, tt], F16, tag="vt")
            nc.scalar.copy(vt_sb[:], vt_ps[:])
            we_ps = ps_vw.tile([M, tt], FP, tag="vw", name="we_ps")
            nc.tensor.matmul(we_ps[:], e80_sb, vt_sb[:], start=True, stop=True)
            we_sb = lsc_pool.tile([M, tt], F16, tag="we", name="we_sb")
            nc.scalar.copy(we_sb[:], we_ps[:])

            lsc_sb = lsc_pool.tile([M, tt], F16, tag="lsc", name="lsc_sb")
            nc.vector.tensor_tensor(lsc_sb[:], low_ps[:], we_sb[:], A.mult)
            return lsc_sb

        def emit_finals(t, off, tt, lsc_sb, base_sb):
            # out[tok, :] = (lsc^T @ B_all) + base, stored fp16 per 128-token
            # chunk; epilogue alternates DVE-direct and ACT-copy + DVE-add.
            nch = tt // 128
            for c in range(nch):
                o_sb = out_pool.tile([128, D], F16, tag="o_sb", name="o_sb")
                for db in range(ND):
                    o_ps = ps_out.tile([128, 512], FP, tag="o_ps")
                    nc.tensor.matmul(
                        o_ps[:],
                        lsc_sb[:, c * 128 : (c + 1) * 128],
                        bb_sb[:, db * 512 : (db + 1) * 512],
                        start=True, stop=True,
                    )
                    if (c + db) % 2 == 0:
                        nc.vector.tensor_tensor(
                            o_sb[:, db * 512 : (db + 1) * 512],
                            o_ps[:],
                            base_sb[:, c, db * 512 : (db + 1) * 512],
                            A.add,
                        )
                    else:
                        cmb_sb = cmb_pool.tile(
                            [128, 512], F16, tag="cmb", name="cmb_sb"
                        )
                        nc.scalar.copy(cmb_sb[:], o_ps[:])
                        nc.vector.tensor_tensor(
                            o_sb[:, db * 512 : (db + 1) * 512],
                            cmb_sb[:],
                            base_sb[:, c, db * 512 : (db + 1) * 512],
                            A.add,
                        )
                nc.gpsimd.dma_start(
                    out_d.ap()[
                        (off + c * 128) * D : (off + (c + 1) * 128) * D
                    ].rearrange("(p d) -> p d", p=128),
                    o_sb[:],
                )

        offs = [sum(TTS[:i]) for i in range(len(TTS))]
        emit_w1_half(0)
        cur = emit_loads(0, offs[0], TTS[0])
        emit_w1_half(1)
        emit_a()
        pending = None
        for t in range(len(TTS)):
            if pending is not None:
                emit_finals(*pending)
            nxt = (
                emit_loads(t + 1, offs[t + 1], TTS[t + 1])
                if t + 1 < len(TTS)
                else None
            )
            lsc_sb = emit_router(t, TTS[t], cur[0])
            pending = (t, offs[t], TTS[t], lsc_sb, cur[1])
            cur = nxt
        emit_finals(*pending)

    nc.compile()
    return nc


def _host_prep(x, base_output, A, B, W1, b1, W2, b2, n_cores=N_CORES,
               scaling=SCALING):
    Bb, S_, Dd = x.shape
    E_, _, R_ = A.shape
    N = Bb * S_
    TOKc = N // n_cores
    KD = Dd // 128
    KH = W1.shape[1] // 128
    M = E_ * R_
    NCHM = max(TTS) // 128
    xf = np.asarray(x, np.float32).reshape(N, Dd).astype(np.float16)
    bf = np.asarray(base_output, np.float32).reshape(N, Dd).astype(np.float16)
    a_all = A.transpose(1, 0, 2).reshape(Dd, M)
    a_all = np.ascontiguousarray(
        a_all.reshape(KD, 128, M).transpose(1, 0, 2).reshape(128, -1),
        np.float16)
    b_all = np.ascontiguousarray(B.reshape(M, Dd) * scaling, np.float16)
    b1v = np.asarray(b1, np.float32).reshape(KH, 128).T
    b2b = np.broadcast_to(
        np.tile(np.asarray(b2, np.float32), NCHM)[None, :], (128, NCHM * E_)
    )
    sm32 = np.ascontiguousarray(np.concatenate([b1v, b2b], axis=1), np.float32)
    w2p = (np.asarray(W2, np.float32)
           .reshape(KH, 128, E_).transpose(1, 0, 2).reshape(128, KH * E_))
    e80 = np.zeros((128, M), np.float32)
    for e in range(E_):
        e80[e, e * R_ : (e + 1) * R_] = 1.0
    ident = np.eye(128, dtype=np.float32)
    sm16 = np.ascontiguousarray(
        np.concatenate([w2p, e80, ident], axis=1), np.float16
    )
    shared = {
        "a_all": a_all,
        "b_all": b_all,
        "w1": np.ascontiguousarray(
            np.asarray(W1, np.float32).reshape(KD, 128, -1)
            .transpose(1, 0, 2).reshape(128, -1)).astype(np.float16),
        "sm32": sm32,
        "sm16": sm16,
    }
    offs = [sum(TTS[:i]) for i in range(len(TTS))]
    in_maps = []
    for i in range(n_cores):
        m = dict(shared)
        xc = xf[i * TOKc : (i + 1) * TOKc]
        bc = bf[i * TOKc : (i + 1) * TOKc]
        xts, bts = [], []
        for off, tt in zip(offs, TTS):
            xts.append(
                xc[off : off + tt].reshape(tt, KD, 128)
                .transpose(2, 1, 0).reshape(-1)
            )
            bts.append(
                bc[off : off + tt].reshape(tt // 128, 128, Dd)
                .transpose(1, 0, 2).reshape(-1)
            )
        m["xt"] = np.ascontiguousarray(np.concatenate(xts))
        m["base"] = np.ascontiguousarray(np.concatenate(bts))
        in_maps.append(m)
    return in_maps, (N, TOKc, Dd)


_NC_CACHE = {}


def _get_nc():
    if "nc" not in _NC_CACHE:
        _NC_CACHE["nc"] = _build_nc()
    return _NC_CACHE["nc"]


def kernel(x, base_output, A, B, W1, b1, W2, b2, _trace=False):
    x = np.asarray(x)
    base_output = np.asarray(base_output)
    nc = _get_nc()
    in_maps, (N, TOKc, Dd) = _host_prep(
        np.asarray(x, np.float32), np.asarray(base_output, np.float32),
        np.asarray(A, np.float32), np.asarray(B, np.float32),
        np.asarray(W1, np.float32), np.asarray(b1, np.float32),
        np.asarray(W2, np.float32), np.asarray(b2, np.float32),
    )
    res = run_bass_kernel_spmd(
        nc, in_maps, core_ids=list(range(N_CORES)), trace=_trace
    )
    outs = []
    for i in range(N_CORES):
        # stores are plain token-major: row = off + c*128 + p
        oc = res.results[i]["out"].reshape(TOKc, Dd)
        outs.append(oc)
    out = np.concatenate(outs, axis=0).astype(np.float32)
    out = out.reshape(x.shape)
    if _trace:
        kernel._last_exec_time_ns = res.exec_time_ns
        kernel._last_results = res
    return out


# revision 11
# speedup vs baseline: 1.0151x; 1.0151x over previous
"""MoLoRA (top-2 MoE LoRA routing) Trainium2 kernel — fp16 data-path version.

Full inputs -> shard tokens across 8 NeuronCores -> Bass/Tile kernel per core
-> gather full output.

Math (per token):
  logits = silu(x @ W1 + b1) @ W2 + b2
  top-2 renormalized softmax weights: w_top1 = sigmoid(l1 - l2),
  w_top2 = 1 - w_top1 (exact identity — no exp/renorm needed).
  combined = sum_e w_e * (x @ A_e @ B_e) * 2.0 ; out = base + combined.

Kernel strategy per core (2048 tokens, fp16 on the wire):
  - x is pre-transposed on the HOST into xT tiles [128 D-part, KD, TT] so no
    on-chip transposes are needed; contractions over D run at full PE rate.
  -

    Variable tile sizes (256,256,512,512,512 tokens): small leading tiles get
    compute and output stores started early so HBM stays saturated; DMA rings
    are specialized (sync: A,W1,xT loads; scalar: B,base loads; gpsimd:
    packed small constants + output stores).
  - Router mm1 in token-on-free layout; logits are produced directly
    token-major by a second matmul with hs chunks as the stationary operand.
  - Top-2 weights via max / masked-second-max / sigmoid(delta).
  - Selected-expert weights expand to the stacked expert-rank dim [80] with a
    0/1 matmul; lowT = A_all^T @ xT is scaled by them; combined output is
    lsc^T @ B_all (B pre-scaled by 2.0 on host) + base, written as fp16 and
    widened to fp32 on the host. The +base epilogue alternates DVE-direct
    adds with ACT-copy + DVE-fast-add to balance engine load.
"""
import sys

for _p in ("/opt/trn_rl_repo",):
    if _p not in sys.path:
        sys.path.insert(0, _p)

import numpy as np
from contextlib import ExitStack

import concourse.bass as bass
import concourse.tile as tile
from concourse import bacc, mybir
from concourse.bass_utils import run_bass_kernel_spmd

FP = mybir.dt.float32
F16 = mybir.dt.float16
NEG_BIG = -1e30

N_CORES = 8
B_, S, D = 4, 4096, 2048
E, R, H = 5, 16, 256
SCALING = 32.0 / 16.0
TOK = (B_ * S) // N_CORES
TTS = (256, 512, 512, 512, 256)
assert sum(TTS) == TOK


def _bcast(ap, n):
    """Append a 0-stride free dim of size n to an AP (read broadcast)."""
    a = ap.copy()
    a.ap = a.ap + [[0, n]]
    return a


def _build_nc(TOK=TOK, D=D, H=H, E=E, R=R, n_cores=N_CORES):
    from concourse.alu_op_type import AluOpType as A

    KD = D // 128
    KH = H // 128
    M = E * R
    ND = D // 512
    TTMAX = max(TTS)

    nc = bacc.Bacc("TRN2", num_devices=n_cores, debug=False)

    xt_d = nc.dram_tensor("xt", [TOK * D], F16, kind="ExternalInput")
    base_d = nc.dram_tensor("base", [TOK * D], F16, kind="ExternalInput")
    a_d = nc.dram_tensor("a_all", [128, KD * M], F16, kind="ExternalInput")
    b_d = nc.dram_tensor("b_all", [M, D], F16, kind="ExternalInput")
    # W1 packed as KH contiguous half-blocks: rows h*128+p, row = KD*128
    w1_d = nc.dram_tensor("w1", [KH * 128, KD * 128], F16, kind="ExternalInput")
    # packed small constants: f32 [128, KH + maxNCH*E] = b1 | b2-broadcast
    NCHM = TTMAX // 128
    sm32_d = nc.dram_tensor("sm32", [128, KH + NCHM * E], FP, kind="ExternalInput")
    # packed small constants: f16 [128, KH*E + M + 128] = w2 | e80 | ident
    sm16_d = nc.dram_tensor(
        "sm16", [128, KH * E + M + 128], F16, kind="ExternalInput"
    )
    out_d = nc.dram_tensor("out", [TOK * D], F16, kind="ExternalOutput")

    with tile.TileContext(nc) as tc, ExitStack() as ctx:
        const = ctx.enter_context(tc.tile_pool(name="const", bufs=1))
        xt_pool = ctx.enter_context(tc.tile_pool(name="xt", bufs=4))
        base_pool = ctx.enter_context(tc.tile_pool(name="basep", bufs=3))
        out_pool = ctx.enter_context(tc.tile_pool(name="outp", bufs=6))
        cmb_pool = ctx.enter_context(tc.tile_pool(name="cmb", bufs=4))
        zs_pool = ctx.enter_context(tc.tile_pool(name="zs", bufs=2))
        sm_pool = ctx.enter_context(tc.tile_pool(name="sm", bufs=2))
        lsc_pool = ctx.enter_context(tc.tile_pool(name="lsc", bufs=2))

        ps_h = ctx.enter_context(tc.tile_pool(name="ps_h", bufs=4, space="PSUM"))
        ps_low = ctx.enter_context(tc.tile_pool(name="ps_low", bufs=1, space="PSUM"))
        ps_out = ctx.enter_context(tc.tile_pool(name="ps_out", bufs=3, space="PSUM"))

        # sync ring order: W1-half0, xt0, W1-half1, A, xt1, ...
        w1h = [const.tile([128, KD, 128], F16, name=f"w1h{h}") for h in range(KH)]
        a_sb = const.tile([128, KD, M], F16)

        def emit_w1_half(h):
            nc.sync.dma_start(
                w1h[h][:],
                w1_d.ap()[h * 128 : (h + 1) * 128, :].rearrange(
                    "p (k j) -> p k j", j=128
                ),
            )

        def emit_a():
            nc.sync.dma_start(
                a_sb[:], a_d.ap().rearrange("p (k m) -> p k m", m=M)
            )
        # scalar ring: B then base tiles
        bb_sb = const.tile([M, D], F16)
        nc.scalar.dma_start(bb_sb[:], b_d.ap())
        # gpsimd ring: packed smalls, then output stores
        sm32_sb = const.tile([128, KH + NCHM * E], FP)
        nc.gpsimd.dma_start(sm32_sb[:], sm32_d.ap())
        sm16_sb = const.tile([128, KH * E + M + 128], F16)
        nc.gpsimd.dma_start(sm16_sb[:], sm16_d.ap())

        b1_sb = sm32_sb[:, 0:KH]
        b2b_full = sm32_sb[:, KH : KH + NCHM * E]
        w2_sb = sm16_sb[:, 0 : KH * E].rearrange("p (k e) -> p k e", e=E)
        e80_sb = sm16_sb[0:E, KH * E : KH * E + M]
        ident = sm16_sb[:, KH * E + M :]

        def emit_loads(t, off, tt):
            nch = tt // 128
            xt_sb = xt_pool.tile([128, KD, tt], F16, tag="xt_sb", name="xt_sb")
            nc.sync.dma_start(
                xt_sb[:],
                xt_d.ap()[off * D : (off + tt) * D].rearrange(
                    "(p k j) -> p k j", p=128, k=KD
                ),
            )
            base_sb = base_pool.tile(
                [128, nch, D], F16, tag="base_sb", name="base_sb"
            )
            nc.scalar.dma_start(
                base_sb[:],
                base_d.ap()[off * D : (off + tt) * D].rearrange(
                    "(p c d) -> p c d", p=128, c=nch
                ),
            )
            return xt_sb, base_sb

        def emit_router(t, tt, xt_sb):
            nch = tt // 128
            # mm1 h-outer (h=0 usable as soon as W1-half0 + xt land)
            h_ps = [
                ps_h.tile([128, tt], FP, tag="hps", name=f"h_ps{h}")
                for h in range(KH)
            ]
            for h in range(KH):
                for k in range(KD):
                    nc.tensor.matmul(
                        h_ps[h][:],
                        w1h[h][:, k, :],
                        xt_sb[:, k, :],
                        start=(k == 0),
                        stop=(k == KD - 1),
                    )
            low_ps = ps_low.tile([M, tt], FP, tag="low")
            for k in range(KD):
                nc.tensor.matmul(
                    low_ps[:],
                    a_sb[:, k, :],
                    xt_sb[:, k, :],
                    start=(k == 0),
                    stop=(k == KD - 1),
                )

            # silu(h + b1) = (h+b1) * sigmoid(h+b1): sg on ACT, fused mult+bias
            # on DVE via scalar_tensor_tensor
            sg_sb = zs_pool.tile([128, KH, tt], F16, tag="sg", name="sg_sb")
            hs_sb = zs_pool.tile([128, KH, tt], F16, tag="hs", name="hs_sb")
            for h in range(KH):
                nc.scalar.activation(
                    sg_sb[:, h, :], h_ps[h][:],
                    mybir.ActivationFunctionType.Sigmoid,
                    bias=b1_sb[:, h : h + 1], scale=1.0,
                )
                nc.vector.scalar_tensor_tensor(
                    hs_sb[:, h, :], h_ps[h][:], b1_sb[:, h : h + 1],
                    sg_sb[:, h, :], op0=A.add, op1=A.mult,
                )

            # logits token-major: lg[tok, e] = sum_h hs[:,h,tokblk]^T @ W2[h]
            lg_ps = ps_out.tile([128, nch, E], FP, tag="o_ps", name="lg_ps")
            for c in range(nch):
                for h in range(KH):
                    nc.tensor.matmul(
                        lg_ps[:, c, :],
                        hs_sb[:, h, c * 128 : (c + 1) * 128],
                        w2_sb[:, h, :],
                        start=(h == 0),
                        stop=(h == KH - 1),
                    )

            # top-2 weights: w1 = sigmoid(m1-m2) for argmax, 1-w1 for argmax2
            b2b_sb = b2b_full[:, 0 : nch * E].rearrange("p (c e) -> p c e", e=E)
            Ls = sm_pool.tile([128, nch, E], FP, tag="Ls")
            nc.vector.tensor_tensor(Ls[:], lg_ps[:], b2b_sb, A.add)
            m1r = sm_pool.tile([128, nch], FP, tag="m1r")
            nc.vector.tensor_reduce(
                m1r[:], Ls[:], axis=mybir.AxisListType.X, op=A.max
            )
            eq = sm_pool.tile([128, nch, E], FP, tag="eq")
            nc.vector.tensor_tensor(
                eq[:], Ls[:], _bcast(m1r[:], E), A.is_equal
            )
            mk = sm_pool.tile([128, nch, E], FP, tag="mk")
            nc.vector.scalar_tensor_tensor(
                mk[:], eq[:], NEG_BIG, Ls[:], op0=A.mult, op1=A.add
            )
            m2r = sm_pool.tile([128, nch], FP, tag="m2r")
            nc.vector.tensor_reduce(
                m2r[:], mk[:], axis=mybir.AxisListType.X, op=A.max
            )
            delta = sm_pool.tile([128, nch], FP, tag="delta")
            nc.vector.tensor_tensor(delta[:], m1r[:], m2r[:], A.subtract)
            s_sg = sm_pool.tile([128, nch], FP, tag="s_sg")
            nc.scalar.activation(
                s_sg[:], delta[:], mybir.ActivationFunctionType.Sigmoid
            )
            u1 = sm_pool.tile([128, nch], FP, tag="u1")
            nc.vector.tensor_scalar(
                u1[:], s_sg[:], -1.0, 1.0, op0=A.mult, op1=A.add
            )
            u2 = sm_pool.tile([128, nch], FP, tag="u2")
            nc.vector.tensor_scalar(
                u2[:], s_sg[:], 2.0, -1.0, op0=A.mult, op1=A.add
            )
            ge2 = sm_pool.tile([128, nch, E], FP, tag="ge2")
            nc.vector.tensor_tensor(
                ge2[:], Ls[:], _bcast(m2r[:], E), A.is_ge
            )
            t1 = sm_pool.tile([128, nch, E], FP, tag="t1")
            nc.vector.tensor_tensor(t1[:], ge2[:], _bcast(u1[:], E), A.mult)
            t2 = sm_pool.tile([128, nch, E], FP, tag="t2")
            nc.vector.tensor_tensor(t2[:], eq[:], _bcast(u2[:], E), A.mult)
            v = sm_pool.tile([128, nch, E], F16, tag="v")
            nc.vector.tensor_tensor(v[:], t1[:], t2[:], A.add)

            # expand weights to stacked expert-rank dim: vT [E,tt] -> [M,tt]
            vt_ps = ps_out.tile([E, tt], F16, tag="o_ps", name="vt_ps")
            for c in range(nch):
                nc.tensor.transpose(
                    vt_ps[:, c * 128 : (c + 1) * 128], v[:, c, :], ident
                )
            vt_sb = sm_pool.tile([E, tt], F16, tag="vt")
            nc.scalar.copy(vt_sb[:], vt_ps[:])
            we_ps = ps_out.tile([M, tt], FP, tag="o_ps", name="we_ps")
            nc.tensor.matmul(we_ps[:], e80_sb, vt_sb[:], start=True, stop=True)
            we_sb = lsc_pool.tile([M, tt], F16, tag="we", name="we_sb")
            nc.scalar.copy(we_sb[:], we_ps[:])

            lsc_sb = lsc_pool.tile([M, tt], F16, tag="lsc", name="lsc_sb")
            nc.vector.tensor_tensor(lsc_sb[:], low_ps[:], we_sb[:], A.mult)
            return lsc_sb

        def emit_finals(t, off, tt, lsc_sb, base_sb):
            # out[tok, :] = (lsc^T @ B_all) + base, stored fp16 per 128-token
            # chunk; epilogue alternates DVE-direct and ACT-copy + DVE-add.
            nch = tt // 128
            for c in range(nch):
                o_sb = out_pool.tile([128, D], F16, tag="o_sb", name="o_sb")
                for db in range(ND):
                    o_ps = ps_out.tile([128, 512], FP, tag="o_ps")
                    nc.tensor.matmul(
                        o_ps[:],
                        lsc_sb[:, c * 128 : (c + 1) * 128],
                        bb_sb[:, db * 512 : (db + 1) * 512],
                        start=True, stop=True,
                    )
                    if (c + db) % 2 == 0:
                        nc.vector.tensor_tensor(
                            o_sb[:, db * 512 : (db + 1) * 512],
                            o_ps[:],
                            base_sb[:, c, db * 512 : (db + 1) * 512],
                            A.add,
                        )
                    else:
                        cmb_sb = cmb_pool.tile(
                            [128, 512], F16, tag="cmb", name="cmb_sb"
                        )
                        nc.scalar.copy(cmb_sb[:], o_ps[:])
                        nc.vector.tensor_tensor(
                            o_sb[:, db * 512 : (db + 1) * 512],
                            cmb_sb[:],
                            base_sb[:, c, db * 512 : (db + 1) * 512],
                            A.add,
                        )
                nc.gpsimd.dma_start(
                    out_d.ap()[
                        (off + c * 128) * D : (off + (c + 1) * 128) * D
                    ].rearrange("(p d) -> p d", p=128),
                    o_sb[:],
                )

        offs = [sum(TTS[:i]) for i in range(len(TTS))]
        emit_w1_half(0)
        cur = emit_loads(0, offs[0], TTS[0])
        emit_w1_half(1)
        emit_a()
        pending = None
        for t in range(len(TTS)):
            if pending is not None:
                emit_finals(*pending)
            nxt = (
                emit_loads(t + 1, offs[t + 1], TTS[t + 1])
                if t + 1 < len(TTS)
                else None
            )
            lsc_sb = emit_router(t, TTS[t], cur[0])
            pending = (t, offs[t], TTS[t], lsc_sb, cur[1])
            cur = nxt
        emit_finals(*pending)

    nc.compile()
    return nc


def _host_prep(x, base_output, A, B, W1, b1, W2, b2, n_cores=N_CORES,
               scaling=SCALING):
    Bb, S_, Dd = x.shape
    E_, _, R_ = A.shape
    N = Bb * S_
    TOKc = N // n_cores
    KD = Dd // 128
    KH = W1.shape[1] // 128
    M = E_ * R_
    NCHM = max(TTS) // 128
    xf = np.asarray(x, np.float32).reshape(N, Dd).astype(np.float16)
    bf = np.asarray(base_output, np.float32).reshape(N, Dd).astype(np.float16)
    a_all = A.transpose(1, 0, 2).reshape(Dd, M)
    a_all = np.ascontiguousarray(
        a_all.reshape(KD, 128, M).transpose(1, 0, 2).reshape(128, -1),
        np.float16)
    b_all = np.ascontiguousarray(B.reshape(M, Dd) * scaling, np.float16)
    b1v = np.asarray(b1, np.float32).reshape(KH, 128).T
    b2b = np.broadcast_to(
        np.tile(np.asarray(b2, np.float32), NCHM)[None, :], (128, NCHM * E_)
    )
    sm32 = np.ascontiguousarray(np.concatenate([b1v, b2b], axis=1), np.float32)
    w2p = (np.asarray(W2, np.float32)
           .reshape(KH, 128, E_).transpose(1, 0, 2).reshape(128, KH * E_))
    e80 = np.zeros((128, M), np.float32)
    for e in range(E_):
        e80[e, e * R_ : (e + 1) * R_] = 1.0
    ident = np.eye(128, dtype=np.float32)
    sm16 = np.ascontiguousarray(
        np.concatenate([w2p, e80, ident], axis=1), np.float16
    )
    shared = {
        "a_all": a_all,
        "b_all": b_all,
        "w1": np.ascontiguousarray(
            np.asarray(W1, np.float32).reshape(KD, 128, KH, 128)
            .transpose(2, 1, 0, 3).reshape(KH * 128, KD * 128)
        ).astype(np.float16),
        "sm32": sm32,
        "sm16": sm16,
    }
    offs = [sum(TTS[:i]) for i in range(len(TTS))]
    in_maps = []
    for i in range(n_cores):
        m = dict(shared)
        xc = xf[i * TOKc : (i + 1) * TOKc]
        bc = bf[i * TOKc : (i + 1) * TOKc]
        xts, bts = [], []
        for off, tt in zip(offs, TTS):
            xts.append(
                xc[off : off + tt].reshape(tt, KD, 128)
                .transpose(2, 1, 0).reshape(-1)
            )
            bts.append(
                bc[off : off + tt].reshape(tt // 128, 128, Dd)
                .transpose(1, 0, 2).reshape(-1)
            )
        m["xt"] = np.ascontiguousarray(np.concatenate(xts))
        m["base"] = np.ascontiguousarray(np.concatenate(bts))
        in_maps.append(m)
    return in_maps, (N, TOKc, Dd)


_NC_CACHE = {}


def _get_nc():
    if "nc" not in _NC_CACHE:
        _NC_CACHE["nc"] = _build_nc()
    return _NC_CACHE["nc"]


def kernel(x, base_output, A, B, W1, b1, W2, b2, _trace=False):
    x = np.asarray(x)
    base_output = np.asarray(base_output)
    nc = _get_nc()
    in_maps, (N, TOKc, Dd) = _host_prep(
        np.asarray(x, np.float32), np.asarray(base_output, np.float32),
        np.asarray(A, np.float32), np.asarray(B, np.float32),
        np.asarray(W1, np.float32), np.asarray(b1, np.float32),
        np.asarray(W2, np.float32), np.asarray(b2, np.float32),
    )
    res = run_bass_kernel_spmd(
        nc, in_maps, core_ids=list(range(N_CORES)), trace=_trace
    )
    outs = []
    for i in range(N_CORES):
        # stores are plain token-major: row = off + c*128 + p
        oc = res.results[i]["out"].reshape(TOKc, Dd)
        outs.append(oc)
    out = np.concatenate(outs, axis=0).astype(np.float32)
    out = out.reshape(x.shape)
    if _trace:
        kernel._last_exec_time_ns = res.exec_time_ns
        kernel._last_results = res
    return out
